# revision 29
# baseline (speedup 1.0000x reference)
"""LSTM encoder (embedding gather + 512-step LSTM) on 8 TRN2 NeuronCores.

Sharding: data-parallel over batch — each of the 8 cores owns 8 of the 64
sequences end-to-end (embedding table and weights replicated), so the
sequential recurrence needs no cross-core communication.

Per-core Bass/Tile kernel (v3):
  v2 profiling showed the PE idle ~30% and — worse — re-throttling to
  K=4/8 (1.2 GHz) every step because the post-matmul chain region left the
  array sparse, so ~7 of 16 h@W_hh groups ran at half clock.  v3 keeps the
  PE stream dense and the evacuation early:

  * hh is emitted bank0-first (all 8 k-tiles of gate-cols 0:512, then
    bank1), so bank0's PSUM->SBUF evacuation + PE transposes + the half-H
    sigmoid/tanh chain overlap the bank1 matmuls instead of trailing them.
  * the x-phase matmuls (xg = X @ W_ih, one m-tile ahead) are interleaved
    between hh groups and the step's seeds/transposes — no >200ns PE lull,
    HAM stays at K=8/8.
  * bank1's transposes are deferred into the next step's PE stream (the
    evacuation CAST is still running when this step's PE work ends).
  * xg slices travel PSUM -> SBUF (cast) -> one SBUF->SBUF DMA straight
    into the step-row layout (tokens are b-major per m-tile, which makes
    the reshape a single linear-partition DMA) — the v2 HBM round trip and
    its just-in-time stalls are gone.
All matmuls are bf16 with fp32 PSUM accumulation; the cell state is fp32.
"""
import sys

if "/opt/trn_rl_repo" not in sys.path:
    sys.path.insert(0, "/opt/trn_rl_repo")

import numpy as np
import ml_dtypes
import concourse.bass as bass
import concourse.tile as tile
from concourse import bacc, mybir
from concourse.masks import make_identity

F32 = mybir.dt.float32
BF16 = mybir.dt.bfloat16
I32 = mybir.dt.int32
P = 128
GATE_PERM = [0, 1, 3, 2]  # strip j -> original gate block (W order: i, f, g, o)

# Problem constants (hardcoded per contest contract)
VOCAB, E, H = 32000, 1024, 1024
B, S = 64, 512
NCORES = 8
BLOC = B // NCORES
U = 16

_program_cache = {}


def build_program(S=S, BLOC=BLOC, E=E, H=H, VOCAB=VOCAB, U=U):
    KT = E // P
    KTH = H // P
    GN = 4 * H
    TOK = S * BLOC
    NIT = S // U          # m-tiles
    NB = NIT // 2         # hardware-loop bodies (2 m-tiles each)
    JB = 4 * BLOC
    assert U == 16 and S % (2 * U) == 0 and TOK // P == NIT

    nc = bacc.Bacc(None, target_bir_lowering=False, debug=False)

    src_idx = nc.dram_tensor("src_idx", [TOK + 2 * P, 1], I32, kind="ExternalInput")
    emb = nc.dram_tensor("emb", [VOCAB, E], BF16, kind="ExternalInput")
    wih = nc.dram_tensor("wih", [P, KT, GN], BF16, kind="ExternalInput")
    whh = nc.dram_tensor("whh", [P, KTH, GN], BF16, kind="ExternalInput")
    bias_rows = nc.dram_tensor("bias_rows", [4, U * H], BF16, kind="ExternalInput")
    scat = nc.dram_tensor("scat", [JB + 4, P], BF16, kind="ExternalInput")
    hs = nc.dram_tensor("hs", [S, P, BLOC * KTH], BF16, kind="ExternalOutput")

    with tile.TileContext(nc) as tc:
        with tc.tile_pool(name="const", bufs=1) as const, \
             tc.tile_pool(name="rw", bufs=1) as rw, \
             tc.tile_pool(name="state", bufs=1) as state, \
             tc.tile_pool(name="rsb", bufs=1) as rsb, \
             tc.tile_pool(name="rps", bufs=2, space="PSUM") as rps, \
             tc.tile_pool(name="gtps_pool", bufs=1, space="PSUM") as gtps_pool, \
             tc.tile_pool(name="xtp", bufs=1, space="PSUM") as xtp, \
             tc.tile_pool(name="xgp", bufs=1, space="PSUM") as xgp:
            ident = const.tile([P, P], BF16)
            make_identity(nc, ident[:])
            whh_sb = rw.tile([P, KTH, GN], BF16)
            nc.sync.dma_start(out=whh_sb[:], in_=whh[:])
            wih_sb = rw.tile([P, KT, GN], BF16)
            nc.sync.dma_start(out=wih_sb[:], in_=wih[:])
            scat_sb = rw.tile([JB + 4, P], BF16)
            nc.sync.dma_start(out=scat_sb[:], in_=scat[:])

            hT = [state.tile([P, KTH * 32], BF16, tag=f"hT{i}", name=f"hT{i}")
                  for i in range(2)]
            cst = [state.tile([P, BLOC * KTH], F32, tag=f"cst{i}", name=f"cst{i}")
                   for i in range(2)]
            nc.vector.memset(hT[0][:], 0.0)
            nc.vector.memset(hT[1][:], 0.0)
            nc.vector.memset(cst[0][:], 0.0)
            # double-buffered per-m-tile xg staging (parity = m-tile index % 2)
            xg_it = [state.tile([JB + 4, U * H], BF16, tag=f"xgit{i}", name=f"xgit{i}")
                     for i in range(2)]
            for i in range(2):
                nc.sync.dma_start(out=xg_it[i][JB:JB + 4, :], in_=bias_rows[:])
            xt_sb = [state.tile([P, KT * P], BF16, tag=f"xt{i}", name=f"xt{i}") for i in range(2)]
            idx_sb = state.tile([P, 1], I32, tag="idx")
            xrow = state.tile([P, E], BF16, tag="xrow")

            # ---------------- x-phase pieces ----------------
            def x_prep_dma(us, mt):
                """DMA slice of gather prep for m-tile mt (int or scalar expr):
                us=0 loads the index column, us=1 fires the gather."""
                if us == 0:
                    nc.sync.dma_start(out=idx_sb[:],
                                      in_=src_idx[bass.ds(mt * P, P), :])
                elif us == 1:
                    nc.gpsimd.indirect_dma_start(
                        out=xrow[:], out_offset=None, in_=emb[:],
                        in_offset=bass.IndirectOffsetOnAxis(ap=idx_sb[:, :1], axis=0))

            def x_prep_tr(us, par):
                """One X.T strip (PE transpose + DVE evac) for slots us=2..9."""
                if not (2 <= us <= 9):
                    return
                c = us - 2
                xt_ps = xtp.tile([P, P], BF16, tag="xtps")
                nc.tensor.transpose(out=xt_ps[:],
                                    in_=xrow[:, c * P:(c + 1) * P],
                                    identity=ident[:])
                nc.vector.tensor_copy(out=xt_sb[par][:, c * P:(c + 1) * P],
                                      in_=xt_ps[:])

            # xg quads: quad q (0..15) = k-range [4*(q%2), 4*(q%2)+4) of slice
            # jn=q//2.  Slot us (0..11) processes QUADS_AT[us] quads; 2-quad
            # slots always hold one FULL slice so a slice's start=True matmul
            # is never emitted in the same step as the previous slice's PSUM
            # evacuation (single xgp bank).  The last slice lands ~4 steps
            # before the next m-tile's first seed needs it.
            QUADS_AT = [1, 1, 2, 1, 1, 2, 1, 1, 2, 1, 1, 2, 0, 0, 0, 0]
            QUAD_OFF = [sum(QUADS_AT[:i]) for i in range(U + 1)]
            xg_acc = {}

            def x_quad(q, par_src):
                """One 4-matmul quad of xg accumulation; on slice completion
                cast PSUM->SBUF and DMA into xg_it (dest parity 1-par_src...
                dest is the buffer the NEXT m-tile reads = opposite of the
                one being consumed now)."""
                jn, ks = q // 2, 4 * (q % 2)
                if ks == 0:
                    xg_acc[jn] = xgp.tile([P, 512], F32, tag="xgps", name="xgps")
                xg_ps = xg_acc[jn]
                for k in range(ks, ks + 4):
                    nc.tensor.matmul(
                        out=xg_ps[:], lhsT=xt_sb[par_src][:, k * P:(k + 1) * P],
                        rhs=wih_sb[:, k, jn * 512:(jn + 1) * 512],
                        start=(k == 0), stop=(k == KT - 1))

            def x_fin(q, par_dst):
                """Evacuate slice jn=q//2 if quad q completed it."""
                if q % 2 != 1:
                    return
                jn = q // 2
                xg_ps = xg_acc.pop(jn)
                j, nh = jn // 2, jn % 2
                xgq = rsb.tile([P, 512], BF16, tag=f"xgq{jn % 2}",
                               name=f"xgq{jn % 2}")
                nc.vector.tensor_copy(out=xgq[:], in_=xg_ps[:])
                # tokens are b-major per m-tile: xgq partition (b*U+u) maps to
                # xg_it partition j*BLOC+b, free offset u*H + nh*512 — a single
                # linear-partition SBUF->SBUF DMA.
                slt = xg_it[par_dst][j * BLOC:(j + 1) * BLOC, :]
                dst = bass.AP(tensor=slt.tensor, offset=slt.offset + nh * 512,
                              ap=[slt.ap[0], [H, U], [1, 512]])
                nc.sync.dma_start(out=dst, in_=xgq[:])

            # ---------------- recurrence ----------------
            pend = {}

            def seeds_one(g, n):
                """Seed bank n of step g's gate PSUM with xg+bias via scatter."""
                u, par = g % U, (g // U) % 2
                if g not in pend:
                    pend[g] = rps.tile([P, 1024], F32, tag="gps", name="gps")
                g_ps = pend[g]
                nc.tensor.matmul(
                    out=g_ps[:, 512 * n:512 * (n + 1)],
                    lhsT=scat_sb[:, :],
                    rhs=xg_it[par][:, u * H + 512 * n: u * H + 512 * (n + 1)],
                    start=True, stop=True)

            def hh_part(g_ps, h_cur, n, k0, k1):
                """hh matmul groups (4 col-tiles each) for bank n, k in [k0,k1)."""
                for k in range(k0, k1):
                    for j in range(4):
                        nc.tensor.matmul(
                            out=g_ps[32 * j:32 * (j + 1), 512 * n:512 * (n + 1)],
                            lhsT=h_cur[:, 32 * k:32 * (k + 1)],
                            rhs=whh_sb[:, k, j * H + 512 * n: j * H + 512 * (n + 1)],
                            start=False, stop=(k == KTH - 1),
                            tile_position=(0, 32 * j),
                            skip_group_check=True)

            def transpose_bank(gq_n, gt_n):
                for c in range(4):
                    nc.tensor.transpose(out=gt_n[:, c * P:(c + 1) * P],
                                        in_=gq_n[:, c * P:(c + 1) * P],
                                        identity=ident[:])

            def chain_a(nh, gt, c_cur, c_new):
                """Half-H gate activations + cell state update for chunks
                [4*nh, 4*nh+4): everything up to c_new."""
                base = gt[:]

                def gt_src(j0, nj):
                    return bass.AP(tensor=base.tensor,
                                   offset=base.offset + 32 * j0,
                                   ap=[base.ap[0], [32, nj], [P, KTH // 2],
                                       [1, BLOC]])

                cs = slice(32 * nh, 32 * nh + 32)
                s_ifo = rsb.tile([P, 96], F32, tag=f"sifo{nh}", name=f"sifo{nh}")
                nc.scalar.activation(
                    out=s_ifo[:].rearrange("p (j c b) -> p j c b", j=3, c=KTH // 2),
                    in_=gt_src(0, 3),
                    func=mybir.ActivationFunctionType.Sigmoid)
                t_g = rsb.tile([P, 32], F32, tag=f"tg{nh}", name=f"tg{nh}")
                nc.scalar.activation(
                    out=t_g[:].rearrange("p (j c b) -> p j c b", j=1, c=KTH // 2),
                    in_=gt_src(3, 1),
                    func=mybir.ActivationFunctionType.Tanh)
                fc = rsb.tile([P, 32], F32, tag=f"fc{nh}", name=f"fc{nh}")
                nc.vector.tensor_tensor(out=fc[:], in0=c_cur[:, cs],
                                        in1=s_ifo[:, 32:64],
                                        op=mybir.AluOpType.mult)
                ig = rsb.tile([P, 32], F32, tag=f"ig{nh}", name=f"ig{nh}")
                nc.vector.tensor_tensor(out=ig[:], in0=t_g[:], in1=s_ifo[:, 0:32],
                                        op=mybir.AluOpType.mult)
                nc.vector.tensor_tensor(out=c_new[:, cs], in0=fc[:], in1=ig[:],
                                        op=mybir.AluOpType.add)
                return s_ifo

            def chain_b(nh, s_ifo, c_new, h_new):
                """tanh(c) and h for chunks [4*nh, 4*nh+4); h lands directly in
                hT strip layout."""
                cs = slice(32 * nh, 32 * nh + 32)
                t_c = rsb.tile([P, 32], F32, tag=f"tc{nh}", name=f"tc{nh}")
                nc.scalar.activation(out=t_c[:], in_=c_new[:, cs],
                                     func=mybir.ActivationFunctionType.Tanh)
                hT_dst = bass.AP(
                    tensor=h_new.tensor,
                    offset=h_new[:].offset + 32 * (KTH // 2) * nh,
                    ap=[h_new[:].ap[0], [32, KTH // 2], [1, BLOC]])
                nc.vector.tensor_tensor(out=hT_dst, in0=t_c[:].rearrange(
                    "p (c b) -> p c b", c=KTH // 2),
                    in1=s_ifo[:, 64:96].rearrange("p (c b) -> p c b", c=KTH // 2),
                    op=mybir.AluOpType.mult)

            # The whole bank1 tail of step g (T1 transposes, half-1 chain,
            # h->HBM stream) is EMITTED inside step g+1, right after its
            # B0 k0-3 groups.  This keeps per-engine FIFO order aligned with
            # execution order: ACT runs [sig1(g-1) tanhg1 tanhc1 | evac0(g)
            # sig0 tanhg0 tanhc0], so evac0 is never queued behind chain1,
            # and the half-1 chain of g-1 executes under step g's matmuls.
            defer_tail = []

            def flush_tail():
                if not defer_tail:
                    return
                (gq1, gt1, c_cur, c_new, h_new, mt, u) = defer_tail.pop()
                transpose_bank(gq1, gt1)            # T1(g-1)
                s1 = chain_a(1, gt1, c_cur, c_new)
                chain_b(1, s1, c_new, h_new)
                # stream step g-1's h (hT strip layout) straight to HBM on
                # the GPSIMD queue (it idles; Sync carries the xg DMAs).
                nc.gpsimd.dma_start(
                    out=hs[bass.ds(mt * U + u, 1), :, :],
                    in_=bass.AP(tensor=h_new.tensor, offset=h_new[:].offset,
                                ap=[h_new[:].ap[0], [1, 1], [32, KTH], [1, BLOC]]))

            def step(g, mt, mt_prep, par_src, g_end):
                """One LSTM step; g in [0, g_end) is the body-local step index.

                PE stream: [B0 k0-3][T1(g-1)][xq][seed0'][xt-tr][xq][B1 k0-3]
                           [B0 k4-7][B1 k4-7][T0][seed1']
                bank0 stops as early as the h1(g-1) arrival allows, so its
                (serial) evac->transpose->sigmoid/tanh->cell->h chain lands
                before step g+1's first matmul needs h strips 0-3."""
                u = g % U
                h_cur, h_new = hT[g % 2], hT[(g + 1) % 2]
                c_cur, c_new = cst[g % 2], cst[(g + 1) % 2]
                if g not in pend:                   # body start: seed inline
                    seeds_one(g, 0)
                    seeds_one(g, 1)
                g_ps = pend.pop(g)
                gq = [rsb.tile([P, 512], BF16, tag=f"gsb{q}", name=f"gsb{q}")
                      for q in range(2)]
                gt = [gtps_pool.tile([P, 512], BF16, tag=f"gt{n}", name=f"gt{n}")
                      for n in range(2)]
                quads = list(range(QUAD_OFF[u], QUAD_OFF[u + 1]))

                x_prep_dma(u, mt_prep)              # idx (u=0) / gather (u=1)
                hh_part(g_ps, h_cur, 0, 0, 4)
                flush_tail()                        # T1 + chain1 + hs of g-1
                if quads:
                    x_quad(quads[0], par_src)
                if g + 1 < g_end:
                    seeds_one(g + 1, 0)
                x_prep_tr(u, 1 - par_src)           # X.T strip for mt_prep
                if len(quads) > 1:
                    x_quad(quads[1], par_src)
                hh_part(g_ps, h_cur, 1, 0, 4)
                hh_part(g_ps, h_cur, 0, 4, KTH)
                # bank0 stopped: evacuate on ACT immediately.
                nc.scalar.copy(out=gq[0][:], in_=g_ps[:, 0:512])
                hh_part(g_ps, h_cur, 1, 4, KTH)
                transpose_bank(gq[0], gt[0])        # T0 (evac0 done by now)
                s0 = chain_a(0, gt[0], c_cur, c_new)
                if g + 1 < g_end:
                    seeds_one(g + 1, 1)
                chain_b(0, s0, c_new, h_new)
                # bank1 evac on DVE after h0 (two half casts, so the next
                # step's first T1 transposes unblock at the half-way point,
                # and h0 — which gates the next step's first matmul — is
                # never FIFO-blocked behind the evacuation).
                nc.vector.tensor_copy(out=gq[1][:, 0:256],
                                      in_=g_ps[:, 512:768])
                nc.vector.tensor_copy(out=gq[1][:, 256:512],
                                      in_=g_ps[:, 768:1024])
                defer_tail.append((gq[1], gt[1], c_cur, c_new, h_new, mt, u))
                for q in quads:
                    x_fin(q, 1 - ((g // U) % 2))

            def body_half(g_base, mt_cur, mt_prep, g_end):
                """16 steps for m-tile mt_cur; xg(mt_cur+1) is computed from
                xt_sb[1-mt_cur%2] while X.T(mt_prep) goes to xt_sb[mt_cur%2]."""
                for u in range(U):
                    step(g_base + u, mt_cur, mt_prep,
                         par_src=1 - (mt_cur % 2), g_end=g_end)

            # ---------------- prologue ----------------
            # xt(0) -> xt_sb[0]; xg(0) -> xg_it[0]; xt(1) -> xt_sb[1];
            # xg(1) -> xg_it[1]; xt(2) -> xt_sb[0] is prepped inside body 0.
            for us in range(10):
                x_prep_dma(us, 0)
                x_prep_tr(us, 0)
            for q in range(16):
                x_quad(q, 0)
                x_fin(q, 0)
            for us in range(10):
                x_prep_dma(us, 1)
                x_prep_tr(us, 1)
            for q in range(16):
                x_quad(q, 1)
                x_fin(q, 1)

            # 2 m-tiles (32 steps) per hardware-loop body.  REPS=2 (64-step
            # bodies, halved back-edges) measured slightly WORSE (4.34ms vs
            # 4.26ms) — the larger body pays more in-body IRAM refetch than
            # it saves in back-edge barriers.
            REPS = 1
            GEND = 2 * U * REPS
            with tc.For_i(0, NB // REPS, 1, hint_engines=(
                    mybir.EngineType.PE, mybir.EngineType.Activation,
                    mybir.EngineType.DVE, mybir.EngineType.SP,
                    mybir.EngineType.Pool)) as iv:
                for rep in range(REPS):
                    base = iv * 2 * REPS + rep * 2
                    # m-tile base (xg in buf0): xg(base+1) from xt_sb[1],
                    # prep xt(base+2) -> buf0; then the odd m-tile.
                    body_half((2 * rep) * U, base, base + 2, GEND)
                    body_half((2 * rep + 1) * U, base + 1, base + 3, GEND)
                flush_tail()                        # last step's bank1 tail
                pend.clear()

    nc.compile()
    return nc


def _prep_inputs(source, embedding, W_ih, W_hh, b, core, n_cores=NCORES):
    src_k = np.asarray(source[core * BLOC:(core + 1) * BLOC, :], dtype=np.int32)
    # b-major token order within each m-tile: idx[mt*128 + b*U + u]
    idx = np.ascontiguousarray(
        src_k.reshape(BLOC, S // U, U).transpose(1, 0, 2).reshape(-1, 1))
    idx = np.concatenate([idx, np.zeros((2 * P, 1), np.int32)], axis=0)  # slack

    def prep_w(W, K):
        Wr = np.asarray(W, np.float32).reshape(K // P, P, 4, H)[:, :, GATE_PERM, :]
        return np.ascontiguousarray(
            Wr.transpose(1, 0, 2, 3).reshape(P, K // P, 4 * H)).astype(ml_dtypes.bfloat16)

    bias_dev = np.ascontiguousarray(
        np.asarray(b, np.float32).reshape(4, H)[GATE_PERM].reshape(4 * H))
    bias_rows = np.tile(bias_dev.reshape(4, H), (1, U))
    JB = 4 * BLOC
    scat = np.zeros((JB + 4, P), np.float32)
    for j in range(4):
        for bb in range(BLOC):
            scat[j * BLOC + bb, 32 * j + bb] = 1.0
            scat[JB + j, 32 * j + bb] = 1.0  # bias row feeds gate strip j
    return {
        "src_idx": idx,
        "emb": np.asarray(embedding, np.float32).astype(ml_dtypes.bfloat16),
        "wih": prep_w(W_ih, E),
        "whh": prep_w(W_hh, H),
        "bias_rows": bias_rows.astype(ml_dtypes.bfloat16),
        "scat": scat.astype(ml_dtypes.bfloat16),
    }


def _unpack_output(hs_dev):
    KTH = H // P
    a = np.asarray(hs_dev, dtype=np.float32).reshape(S, P, KTH, BLOC)
    return np.ascontiguousarray(a.transpose(3, 0, 2, 1)).reshape(BLOC, S, H)


def _get_program():
    if "nc" not in _program_cache:
        _program_cache["nc"] = build_program()
    return _program_cache["nc"]


def kernel(source, embedding, W_ih, W_hh, b):
    """Full inputs in, full output out. Shards batch over 8 NeuronCores."""
    from concourse import bass2jax

    source = np.asarray(source)
    embedding = np.asarray(embedding, np.float32)
    W_ih = np.asarray(W_ih, np.float32)
    W_hh = np.asarray(W_hh, np.float32)
    b = np.asarray(b, np.float32)

    nc = _get_program()
    in_maps = [_prep_inputs(source, embedding, W_ih, W_hh, b, core=k)
               for k in range(NCORES)]
    res = bass2jax.run_bass_via_pjrt(nc, in_maps, n_cores=NCORES)
    out = np.concatenate([_unpack_output(res[k]["hs"]) for k in range(NCORES)],
                         axis=0)
    return out.astype(np.float32)
